# revision 83
# baseline (speedup 1.0000x reference)
"""Multi-head causal attention (B=2, S=2048, D=1024, H=16) on 8 TRN2 NeuronCores.

Sharding: tensor-parallel over heads. Core c owns heads [2c, 2c+1]:
  - Wq/Wk/Wv column-shard [1024, 128] (2 heads x 64)
  - Wo row-shard [128, 1024]
Each core computes a partial output [2, 2048, 1024]; host sums partials + bo.

Host marshalling (free): X is pre-transposed to XT[b, d, s] in bf16 and the
weights pre-packed, so the kernel never transposes X on device.

Per-core pipeline (all matmul streams bf16, PSUM f32):
  proj:  QT/KT = W^T X^T   [128(2h x 64), 2048]  (stationary W, moving XT)
         V natural per s-block of 128 (stationary XT chunk, moving Wv),
         stored as [V_h0 | 1 | V_h1 | 1] with ones columns for denominators
  attn per 512-wide q wave:
         scores[k, q] = KT^T QT (64-contraction per head; emitted at high
         priority so the scalar engine's exp stream never starves)
         et = exp(0.125 * scores) on ACT; causal mask on diagonal 128-blocks
         applied post-exp on GpSimd (affine_select)
         PV per 128-q block: ctx_nat[q, 65] += et_block^T @ [V_h | 1]
         (stationary et -> streams only 65 columns; col 64 = denominator)
         normalize: reciprocal + broadcast multiply (DVE)
         ctxT via PE transpose, then out[s, :] = ctxT^T @ Wo

Scheduling (emission order = scheduler priority = psum ring-slot order):
  PE warmup dummies cover the initial DMA wait (p-state ramp); b1's
  projection and b0's deferred out-projections fill PE gaps inside the
  ACT-bound attention spans; late-phase drains ride the scalar engine and
  tail out-projections borrow the then-idle scores psum ring.
"""

import numpy as np

B, S, D = 2, 2048, 1024
H_PER_CORE = 2
HD = 64
DM = H_PER_CORE * HD   # 128, per-core model-dim shard
N_CORES = 8
P = 128
KO = D // P            # 8 contraction chunks
NSJ = S // 512         # 4 q waves of 512
NSI = S // P           # 16 s-blocks of 128

_BUILD_CACHE = {}


def build_bass(mm_mode: str = "fp32r"):
    """Build the per-core Bass program. mm_mode kept for test harness compat."""
    import contextlib

    import concourse.bass as cbass
    import concourse.tile as tile
    from concourse import bacc, mybir
    from concourse.masks import make_identity

    f32 = mybir.dt.float32
    f16 = mybir.dt.float16
    bf16 = mybir.dt.bfloat16
    Exp = mybir.ActivationFunctionType.Exp

    nc = bacc.Bacc("TRN2", target_bir_lowering=False, debug=False)

    XT = nc.dram_tensor("XT", [B, D, S], bf16, kind="ExternalInput").ap()
    Wq = nc.dram_tensor("Wq", [P, KO, DM], bf16, kind="ExternalInput").ap()
    Wk = nc.dram_tensor("Wk", [P, KO, DM], bf16, kind="ExternalInput").ap()
    Wv = nc.dram_tensor("Wv", [P, KO, DM], bf16, kind="ExternalInput").ap()
    Wo = nc.dram_tensor("Wo", [DM, D], bf16, kind="ExternalInput").ap()
    Out = nc.dram_tensor("Out", [B, S, D], f16, kind="ExternalOutput").ap()

    lp_ctx = (nc.allow_low_precision(reason="bf16 throughput is intentional")
              if hasattr(nc, "allow_low_precision") else contextlib.nullcontext())
    with lp_ctx, tile.TileContext(nc) as tc:
        with tc.tile_pool(name="consts", bufs=1) as consts, \
             tc.tile_pool(name="wpool", bufs=1) as wpool, \
             tc.tile_pool(name="xt", bufs=20) as xtp, \
             tc.tile_pool(name="qk", bufs=2) as qkp, \
             tc.tile_pool(name="vp", bufs=40) as vpp, \
             tc.tile_pool(name="et", bufs=56) as etp, \
             tc.tile_pool(name="cn", bufs=12) as cnp, \
             tc.tile_pool(name="rc", bufs=12) as rcp, \
             tc.tile_pool(name="ct", bufs=12) as ctp, \
             tc.tile_pool(name="ot", bufs=10) as otp, \
             tc.tile_pool(name="psum", bufs=2, space="PSUM") as psum:

            # ---- constants ----
            ident_f32 = consts.tile([P, P], f32, tag="ident_f32")
            make_identity(nc, ident_f32[:])
            ident = consts.tile([P, P], bf16, tag="ident")
            nc.vector.tensor_copy(out=ident[:], in_=ident_f32[:])

            # PE warmup: dummy transposes fill the initial DMA wait so the
            # p-state ramp (3us of continuous busy) completes before real
            # matmuls start
            for _ in range(10):
                wps = psum.tile([P, P], f32, tag="s", name="wps",
                                padded_shape=[P, 512])
                nc.tensor.matmul(wps[:], ident_f32[:], ident_f32[:],
                                 is_transpose=True, skip_group_check=True)

            # ---- weights + XT loads, ordered to minimize time-to-first-MM:
            # Wq, first XT chunk, then the rest ----
            def load_w(ap, name):
                t = wpool.tile([P, KO, DM], bf16, tag=name)
                nc.sync.dma_start(t[:], ap)
                return t

            # XT chunks of [128, 2ko, 512]; 8 per batch
            xts_all = {}

            def load_xt(b, sj, kh):
                t = xtp.tile([P, 2, 512], bf16, tag="xt",
                             name=f"xt{b}_{sj}_{kh}")
                nc.sync.dma_start(
                    t[:],
                    XT[b, kh * 2 * P:(kh * 2 + 2) * P, sj * 512:(sj + 1) * 512]
                    .rearrange("(ko p) s -> p ko s", p=P),
                )
                xts_all[(b, sj, kh)] = t

            Wq_sb = load_w(Wq, "wq")
            for kh in range(4):
                load_xt(0, 0, kh)
            Wk_sb = load_w(Wk, "wk")
            Wv_sb = load_w(Wv, "wv")
            Wo_sb = wpool.tile([DM, D], bf16, tag="wo")
            nc.sync.dma_start(Wo_sb[:], Wo[:])
            for b in range(B):
                for sj in range(NSJ):
                    for kh in range(4):
                        if (b, sj, kh) not in xts_all:
                            load_xt(b, sj, kh)

            def xt_ap(b, sj, ko):
                return xts_all[(b, sj, ko // 2)][:, ko % 2, :]

            QT, KT, V = {}, {}, {}

            def emit_proj_qk(b, sj):
                if sj == 0:
                    QT[b] = qkp.tile([DM, S], bf16, tag="qt", name="QT")
                    KT[b] = qkp.tile([DM, S], bf16, tag="kt", name="KT")
                for w_sb, dst in ((Wq_sb, QT[b]), (Wk_sb, KT[b])):
                    ps = psum.tile([P, 512], f32, tag="p", name="ps_qk")
                    for ko in range(KO):
                        nc.tensor.matmul(
                            ps[:], w_sb[:, ko, :], xt_ap(b, sj, ko),
                            start=(ko == 0), stop=(ko == KO - 1),
                        )
                    nc.vector.tensor_copy(
                        out=dst[:, sj * 512:(sj + 1) * 512], in_=ps[:]
                    )

            def emit_proj_v(b):
                # V natural per s-block, with ones columns:
                # V[si] [128, 130]: 0:64 = h0, col 64 = 1; 65:129 = h1, 129 = 1
                V[b] = []
                for si in range(NSI):
                    ps = psum.tile([P, 512], f32, tag="p", name="ps_v")
                    for ko in range(KO):
                        nc.tensor.matmul(
                            ps[:, 0:DM],
                            xt_ap(b, si // 4, ko)[:, (si % 4) * P:(si % 4 + 1) * P],
                            Wv_sb[:, ko, :],
                            start=(ko == 0), stop=(ko == KO - 1),
                        )
                    v = vpp.tile([P, 2 * (HD + 1)], bf16, tag="v", name="v")
                    vv = v[:, :].rearrange("p (h y) -> p h y", h=2)
                    nc.vector.tensor_copy(
                        out=vv[:, :, 0:HD],
                        in_=ps[:, 0:DM].rearrange("p (h x) -> p h x", h=2),
                    )
                    nc.gpsimd.memset(vv[:, :, HD:HD + 1], 1.0)
                    V[b].append(v)

            CT = {}

            def emit_scores(b, qj):
                nk = 4 * qj + 4
                ets = {}
                for ki in range(nk):
                    col0 = P * (ki - 4 * qj) if ki >= 4 * qj else 0
                    for h in range(H_PER_CORE):
                        hp = slice(h * HD, (h + 1) * HD)
                        sps = psum.tile([P, 512], f32, tag="s", name="sps")
                        with tc.high_priority():
                            nc.tensor.matmul(
                                sps[:, col0:],
                                KT[b][hp, ki * P:(ki + 1) * P],
                                QT[b][hp, qj * 512 + col0:(qj + 1) * 512],
                                start=True, stop=True,
                            )
                        et = etp.tile([P, 512], bf16, tag="et", name="et")
                        nc.scalar.activation(
                            et[:, col0:], sps[:, col0:], Exp, scale=0.125
                        )
                        if ki >= 4 * qj:
                            # zero strictly-upper triangle (k > q)
                            nc.gpsimd.affine_select(
                                out=et[:, col0:col0 + P],
                                in_=et[:, col0:col0 + P],
                                compare_op=mybir.AluOpType.is_ge,
                                fill=0.0, base=0,
                                pattern=[[1, P]],
                                channel_multiplier=-1,
                            )
                        ets[(ki, h)] = et
                return ets

            def emit_pv(b, qj, ets, act_assist=False):
                # PV per 128-q block; stationary et, moving [V_h | 1]
                cns = []
                for r in range(4):
                    qi = 4 * qj + r
                    cps = psum.tile([P, 2 * (HD + 1)], f32, tag="c",
                                    name="cps")
                    for h in range(H_PER_CORE):
                        vs = slice(h * (HD + 1), (h + 1) * (HD + 1))
                        for ki in range(qi + 1):
                            nc.tensor.matmul(
                                cps[:, vs],
                                ets[(ki, h)][:, r * P:(r + 1) * P],
                                V[b][ki][:, vs],
                                start=(ki == 0), stop=(ki == qi),
                                skip_group_check=True,
                            )
                    rc = rcp.tile([P, 2], f32, tag="rc", name="rc")
                    cps_v = cps[:, :].rearrange("p (h y) -> p h y", h=2)
                    nc.vector.reciprocal(rc[:, 0:2], cps_v[:, :, HD])
                    cn = cnp.tile([P, DM], bf16, tag="cn", name="cn")
                    if act_assist:
                        for h in range(H_PER_CORE):
                            nc.scalar.mul(
                                cn[:, h * HD:(h + 1) * HD],
                                cps[:, h * (HD + 1):h * (HD + 1) + HD],
                                rc[:, h:h + 1],
                            )
                    else:
                        src_v = cps_v[:, :, 0:HD]
                        rc_v = rc[:, :].rearrange("p (h o) -> p h o", o=1)
                        src_b, rc_b = cbass.broadcast_tensor_aps(src_v, rc_v)
                        nc.vector.tensor_tensor(
                            cn[:, :].rearrange("p (h x) -> p h x", h=2),
                            src_b, rc_b, mybir.AluOpType.mult,
                        )
                    cns.append(cn)

                # ctxT via PE transpose: [128 dm, 512 q]
                ctps = psum.tile([P, 512], bf16, tag="c", name="ctps",
                                 padded_shape=[P, 1024])
                for r in range(4):
                    nc.tensor.matmul(
                        ctps[:, r * P:(r + 1) * P], cns[r][:], ident[:],
                        is_transpose=True, skip_group_check=True,
                    )
                ct = ctp.tile([P, 512], bf16, tag="ct", name="ct")
                if act_assist:
                    nc.scalar.copy(ct[:], ctps[:])
                else:
                    nc.vector.tensor_copy(out=ct[:], in_=ctps[:])
                CT[(b, qj)] = ct

            def emit_out(b, qj, act_assist, tail=False):
                ct = CT[(b, qj)]
                for r in range(4):
                    ot = otp.tile([P, D], f16, tag="ot", name="ot")
                    for dj in range(2):
                        # after the last exp the "s" ring is free: tail outs
                        # alternate rings for a 4-deep MM/drain pipeline
                        tag = "s" if (tail and dj == 1) else "o"
                        ops = psum.tile([P, 512], f32, tag=tag, name="ops")
                        nc.tensor.matmul(
                            ops[:],
                            ct[:, r * P:(r + 1) * P],
                            Wo_sb[:, dj * 512:(dj + 1) * 512],
                            start=True, stop=True,
                        )
                        if act_assist and dj == 1:
                            nc.scalar.copy(
                                ot[:, dj * 512:(dj + 1) * 512], ops[:]
                            )
                        else:
                            nc.vector.tensor_copy(
                                out=ot[:, dj * 512:(dj + 1) * 512], in_=ops[:]
                            )
                    nc.sync.dma_start(
                        Out[b, (4 * qj + r) * P:(4 * qj + r + 1) * P, :],
                        ot[:],
                    )

            # Emission order = scheduler priority AND pool-ring slot order.
            # Per batch: Q/K proj, then wave-0 scores BEFORE the V
            # projection (feeds ACT early); each wave's out-projection is
            # emitted after the NEXT wave's scores so ACT never starves at
            # wave boundaries. b1's projection (emitted after b0's waves)
            # fills b0's ACT-bound attention gaps via readiness.
            def emit_batch(b, inline_outs, woven_outs):
                emit_proj_qk(b, 0)
                ets0 = emit_scores(b, 0)
                emit_proj_qk(b, 1)
                emit_proj_qk(b, 2)
                emit_proj_qk(b, 3)
                ets1 = emit_scores(b, 1)
                emit_proj_v(b)
                emit_pv(b, 0, ets0)
                if woven_outs:
                    emit_out(*woven_outs[0], act_assist=False)
                ets2 = emit_scores(b, 2)
                emit_pv(b, 1, ets1)
                if inline_outs:
                    emit_out(b, 0, act_assist=False)
                if woven_outs:
                    emit_out(*woven_outs[1], act_assist=False)
                ets3 = emit_scores(b, 3)
                emit_pv(b, 2, ets2)
                if inline_outs:
                    emit_out(b, 1, act_assist=False)
                emit_pv(b, 3, ets3, act_assist=True)

            emit_batch(0, True, [])
            emit_batch(1, False, [(0, 2), (0, 3)])
            emit_out(1, 0, act_assist=False)
            emit_out(1, 1, act_assist=True, tail=True)
            emit_out(1, 2, act_assist=True, tail=True)
            emit_out(1, 3, act_assist=True, tail=True)

    nc.compile()
    return nc


def _get_nc(mm_mode: str = "fp32r"):
    if mm_mode not in _BUILD_CACHE:
        _BUILD_CACHE[mm_mode] = build_bass(mm_mode)
    return _BUILD_CACHE[mm_mode]


def kernel(X, Wq, Wk, Wv, Wo, bo, mm_mode: str = "fp32r"):
    import ml_dtypes
    from concourse.bass_utils import run_bass_kernel_spmd

    bf16 = ml_dtypes.bfloat16
    X = np.asarray(X, dtype=np.float32)
    Wq = np.asarray(Wq, dtype=np.float32)
    Wk = np.asarray(Wk, dtype=np.float32)
    Wv = np.asarray(Wv, dtype=np.float32)
    Wo = np.asarray(Wo, dtype=np.float32)
    bo = np.asarray(bo, dtype=np.float32)

    nc = _get_nc(mm_mode)

    # host marshalling: XT[b, d, s] bf16; W column-shards packed (ko p) m -> p ko m
    XT = np.ascontiguousarray(X.transpose(0, 2, 1)).astype(bf16)

    def pack_w(w, cs):
        return np.ascontiguousarray(
            w[:, cs].reshape(KO, P, DM).transpose(1, 0, 2)
        ).astype(bf16)

    in_maps = []
    for c in range(N_CORES):
        cs = slice(c * DM, (c + 1) * DM)
        in_maps.append({
            "XT": XT,
            "Wq": pack_w(Wq, cs),
            "Wk": pack_w(Wk, cs),
            "Wv": pack_w(Wv, cs),
            "Wo": np.ascontiguousarray(Wo[cs, :]).astype(bf16),
        })

    res = run_bass_kernel_spmd(nc, in_maps, core_ids=list(range(N_CORES)))
    out = np.zeros((B, S, D), dtype=np.float64)
    for c in range(N_CORES):
        out += res.results[c]["Out"].astype(np.float64)
    out += bo.astype(np.float64)
    return out.astype(np.float32)
